# revision 30
# baseline (speedup 1.0000x reference)
"""Autoformer encoder block on 8 TRN2 NeuronCores, v2.

Sharding: data-parallel over batch (B=8 -> 1 batch per core), weights
replicated, no collectives.

Per-core math (S=1024, D=512, H=8, dp=64, K=25):
  trend = movavg(x) via banded [128,128] matmuls on token blocks
  seas = x - trend (bf16), transposed to feature-major seasT
  q/k = wq/wk proj (bf16 GEMMs); v only needs tokens 0:63 + tail sums
  The reference rfft over depth (n=2S) zeroes corr for lag >= dp, so:
    QF/KF/KFswap = stacked 128-pt real DFT (cos|sin packed in 128 rows)
    P1 = QF*KF, P2 = QF*KFswap (2 DVE ops/head-chunk)
    corr = INV1.T@P1 + INV2.T@P2 ; E = exp(corr/8)
    numerator = blockdiag(v).T @ E (head pairs), Z via ones-matmul
    attnT = (nv + Wtail) * broadcast(1/(Z+S-dp))   [bv folded into bo]
  wo + seas residual via identity-matmul into PSUM, LN1, FFN (4x, relu),
  FFN2 + out1 residual via identity-matmul, LN2 (no gamma), then
  tail: token-major PSUM = yhat.T@diag(g2) + I@x + ones@be2  (seas+trend==x),
  LN3 token-major (per-partition scale/bias), g3/be3 broadcast, DMA out.
"""

import numpy as np
import ml_dtypes

B, S, D, H = 8, 1024, 512, 8
DP = D // H
DH = 4 * D
KWIN, PAD = 25, 12
EPS = 1e-6
NCORES = 8
NT = S // 128   # 8 token tiles
ND = D // 128   # 4 feature tiles
NH = DH // 128  # 16 hidden tiles

BF = ml_dtypes.bfloat16
_CACHE = {}
DEBUG = None  # name of intermediate to dump into `out` instead of result


def _np_consts():
    c = {}
    cnt = (np.minimum(S, np.arange(S) + PAD + 1)
           - np.maximum(0, np.arange(S) - PAD)).astype(np.float32)
    A_prev = np.zeros((128, 128), np.float32)
    A_mid = np.zeros((128, 128), np.float32)
    A_next = np.zeros((128, 128), np.float32)
    A_mid0 = np.zeros((128, 128), np.float32)
    A_mid7 = np.zeros((128, 128), np.float32)
    for cc in range(128):
        for i in range(128):
            if abs(i - cc) <= PAD:
                A_mid[i, cc] = 1.0 / 25.0
                A_mid0[i, cc] = 1.0 / cnt[cc]
                A_mid7[i, cc] = 1.0 / cnt[896 + cc]
            if abs(i - 128 - cc) <= PAD:
                A_prev[i, cc] = 1.0 / 25.0
            if abs(i + 128 - cc) <= PAD:
                A_next[i, cc] = 1.0 / 25.0
    c["mav"] = np.concatenate(
        [A_prev, A_mid, A_next, A_mid0, A_mid7], axis=1).astype(BF)

    n = 2 * DP
    dd = np.arange(DP)[:, None]
    f65 = np.arange(65)[None, :]
    COS = np.cos(2 * np.pi * f65 * dd / n)
    SIN = np.sin(2 * np.pi * f65 * dd / n)
    FWD_std = np.concatenate([COS, SIN[:, 1:64]], axis=1)
    FWD_swp = np.concatenate([SIN, COS[:, 1:64]], axis=1)
    fwd1 = np.concatenate([FWD_std, FWD_swp], axis=1)  # [64, 256]
    c["fwd"] = np.concatenate([fwd1, fwd1], axis=0).astype(BF)  # [128, 256]
    t64 = np.arange(DP)[None, :]
    fr = np.arange(65)[:, None]
    w = np.full((65, 1), 2.0); w[0] = 1.0; w[64] = 1.0
    IRE = (w / n) * np.cos(2 * np.pi * fr * t64 / n)
    IIM = -(2.0 / n) * np.sin(2 * np.pi * fr * t64 / n)
    INV1 = np.concatenate([IRE, IRE[1:64]], axis=0)
    INV2 = np.concatenate([IIM, -IIM[1:64]], axis=0)
    c["inv"] = np.concatenate([INV1, INV2], axis=1).astype(BF)  # [128,128]

    c["identb"] = np.eye(128, dtype=BF)
    c["onesb"] = np.ones((128, 128), BF)
    c["onesf"] = np.ones((1, 128), np.float32)
    zsel = np.zeros((128, 32), np.float32)
    for p in range(4):
        zsel[0:64, p * 8 + 2 * p] = 1.0
        zsel[64:128, p * 8 + 2 * p + 1] = 1.0
    c["zsel"] = zsel.astype(BF)
    hsel = np.zeros((8, 4 * 128), np.float32)
    for p in range(4):
        for m in range(128):
            hsel[2 * p + m // 64, p * 128 + m] = 1.0
    c["hsel"] = hsel.astype(BF)
    return c


# vec pack column offsets (each D-vector -> [128, ND] cols; b1 -> 16)
VC_BQ, VC_BK, VC_BO, VC_B2 = 0, 4, 8, 12
VC_G1, VC_BE1, VC_G2, VC_BE2 = 16, 20, 24, 28
VC_B1 = 32
VC_N = 48


def _build():
    import concourse.bacc as bacc
    import concourse.mybir as mybir
    import concourse.tile as tile

    f32 = mybir.dt.float32
    f32r = mybir.dt.float32r
    bf16 = mybir.dt.bfloat16
    Alu = mybir.AluOpType
    Act = mybir.ActivationFunctionType

    nc = bacc.Bacc()

    x_d = nc.declare_dram_parameter("xb", [S, D], bf16, isOutput=False)
    wqk_d = nc.declare_dram_parameter("wqk", [128, 2 * ND * D], bf16, isOutput=False)
    wvo_d = nc.declare_dram_parameter("wvo", [128, 2 * ND * D], bf16, isOutput=False)
    w1_d = nc.declare_dram_parameter("w1p", [128, ND * DH], bf16, isOutput=False)
    w2_d = nc.declare_dram_parameter("w2p", [128, NH * D], bf16, isOutput=False)
    vec_d = nc.declare_dram_parameter("vecs", [128, VC_N], f32, isOutput=False)
    rowf_d = nc.declare_dram_parameter("rowsf", [1, 2 * D], f32, isOutput=False)
    rowb_d = nc.declare_dram_parameter("rowsb", [1, D], bf16, isOutput=False)
    g2d_d = nc.declare_dram_parameter("g2diag", [128, D], bf16, isOutput=False)
    out_d = nc.declare_dram_parameter("out", [S, D], f32, isOutput=True)

    C = _np_consts()
    mav_d = nc.inline_tensor(C["mav"], name="c_mav")
    fwd_d = nc.inline_tensor(C["fwd"], name="c_fwd")
    inv_d = nc.inline_tensor(C["inv"], name="c_inv")
    identb_d = nc.inline_tensor(C["identb"], name="c_identb")
    onesb_d = nc.inline_tensor(C["onesb"], name="c_onesb")
    onesf_d = nc.inline_tensor(C["onesf"], name="c_onesf")
    zsel_d = nc.inline_tensor(C["zsel"], name="c_zsel")
    hsel_d = nc.inline_tensor(C["hsel"], name="c_hsel")

    def r(ap):
        return ap.bitcast(f32r)

    def mm(out, lhsT, rhs, start=True, stop=True):
        nc.tensor.matmul(out, lhsT, rhs, start=start, stop=stop)

    def dump(tc, tile_src, ncols, aoff=0):
        with tc.tile_pool(name="dbg", bufs=2) as dbg:
            for c0 in range(0, ncols, 512):
                t = dbg.tile([128, 512], f32, tag="d")
                nc.vector.tensor_copy(t[:], tile_src[:, c0:c0 + 512])
                nc.sync.dma_start(
                    out_d.rearrange("(a p) d -> p a d", p=128)[
                        :, aoff + c0 // 512, :],
                    t[:])

    CUT = {None: 99, 'seas': 1, 'qT': 2, 'E': 3, 'attnT': 3,
           'o1pre': 4, 'out1': 4, 'y': 4, 'yh': 4, 'xo': 4}[DEBUG]

    with tile.TileContext(nc) as tc:
        with (
            tc.tile_pool(name="konst", bufs=1) as konst,
            tc.tile_pool(name="persist", bufs=1) as persist,
            tc.tile_pool(name="wts", bufs=1) as wts,
        ):
            # ---- consts + inputs to SBUF ----
            x_tok = persist.tile([128, NT * D], bf16)
            for xh_ in range(2):
                nc.sync.dma_start(
                    x_tok[:, xh_ * 4 * D:(xh_ + 1) * 4 * D].rearrange(
                        "p (st d) -> p st d", d=D),
                    x_d[xh_ * 512:(xh_ + 1) * 512, :].rearrange(
                        "(st p) d -> p st d", p=128))
            mav = konst.tile([128, 640], bf16)
            fwd = konst.tile([128, 256], bf16)
            inv = konst.tile([128, 128], bf16)
            identb = konst.tile([128, 128], bf16)
            onesb = konst.tile([128, 128], bf16)
            onesf = konst.tile([1, 128], f32)
            zsel = konst.tile([128, 32], bf16)
            hsel = konst.tile([8, 4 * 128], bf16)
            for tl, dr in ((mav, mav_d), (fwd, fwd_d), (inv, inv_d),
                           (identb, identb_d), (onesb, onesb_d),
                           (zsel, zsel_d), (hsel, hsel_d)):
                nc.sync.dma_start(tl[:], dr[:])
            nc.sync.dma_start(r(onesf[:]), r(onesf_d[:]))
            vec = konst.tile([128, VC_N], f32)
            nc.sync.dma_start(vec[:], vec_d[:])
            rowf = konst.tile([1, 2 * D], f32)
            nc.sync.dma_start(r(rowf[:]), r(rowf_d[:]))
            rowb = konst.tile([1, D], bf16)
            nc.sync.dma_start(rowb[:], rowb_d[:])
            g2diag = konst.tile([128, D], bf16)
            nc.sync.dma_start(g2diag[:], g2d_d[:])
            wqk = wts.tile([128, 2 * ND * D], bf16)
            nc.sync.dma_start(wqk[:], wqk_d[:])
            wvo = wts.tile([128, 2 * ND * D], bf16)
            nc.sync.dma_start(wvo[:], wvo_d[:])
            w1_s = wts.tile([128, ND * DH], bf16)
            nc.sync.dma_start(w1_s[:], w1_d[:])
            w2_s = wts.tile([128, NH * D], bf16)
            nc.sync.dma_start(w2_s[:], w2_d[:])

            seasT = persist.tile([128, ND * S], bf16)
            attnT = persist.tile([128, ND * S], bf16)
            W_sb = persist.tile([128, ND], f32)

            A_prev, A_mid, A_next = mav[:, 0:128], mav[:, 128:256], mav[:, 256:384]
            A_mid0, A_mid7 = mav[:, 384:512], mav[:, 512:640]

            # ============ phase 1: movavg, seas, transpose, W sums ==========
            with (
                tc.tile_pool(name="ph1", bufs=1) as ph1,
                tc.tile_pool(name="sc1", bufs=2) as sc1,
                tc.tile_pool(name="ps1", bufs=2, space="PSUM") as ps1,
                tc.tile_pool(name="ps1t", bufs=4, space="PSUM") as ps1t,
            ):
                seas_tok = ph1.tile([128, NT * D], bf16)
                for j in range(NT):
                    ps = ps1.tile([128, D], f32, tag="mavg")
                    mids = A_mid0 if j == 0 else (A_mid7 if j == 7 else A_mid)
                    pieces = [(mids, j)]
                    if j > 0:
                        pieces.append((A_prev, j - 1))
                    if j < NT - 1:
                        pieces.append((A_next, j + 1))
                    for i, (a_, jj) in enumerate(pieces):
                        mm(ps[:], a_, x_tok[:, jj * D:(jj + 1) * D],
                           start=(i == 0), stop=(i == len(pieces) - 1))
                    nc.vector.tensor_tensor(
                        seas_tok[:, j * D:(j + 1) * D],
                        x_tok[:, j * D:(j + 1) * D], ps[:], Alu.subtract)
                if DEBUG == 'seas':
                    dump(tc, seas_tok, NT * D)
                for st in range(NT):
                    for kt in range(ND):
                        pt = ps1t.tile([128, 128], f32, tag="tr")
                        mm(pt[:], seas_tok[:, st * D + kt * 128:st * D + (kt + 1) * 128],
                           identb[:])
                        dst = seasT[:, kt * S + st * 128:kt * S + (st + 1) * 128]
                        if (st + kt) % 2 == 0:
                            nc.vector.tensor_copy(dst, pt[:])
                        else:
                            nc.scalar.copy(dst, pt[:])

                # tail sums of seas (features x 1) -> W = wv.T @ sdif
                sdif = ph1.tile([128, 2 * ND], f32)
                sdif_b = ph1.tile([128, 2 * ND], bf16)
                dmp = sc1.tile([128, S - DP], f32, tag="dmp")
                for k in range(ND):
                    with nc.allow_low_precision(reason="accum copy"):
                        nc.scalar.activation(
                            dmp[:], seasT[:, k * S + DP:(k + 1) * S],
                            Act.Copy, accum_out=sdif[:, 2 * k:2 * k + 1])
                    nc.vector.tensor_copy(sdif_b[:, 2 * k:2 * k + 1],
                                          sdif[:, 2 * k:2 * k + 1])
                    nc.vector.tensor_copy(sdif_b[:, 2 * k + 1:2 * k + 2],
                                          sdif[:, 2 * k:2 * k + 1])
                with tc.tile_pool(name="psw", bufs=1, space="PSUM") as psw:
                    ps_w = psw.tile([128, 2 * ND], f32)
                    for mt in range(ND):
                        for k in range(ND):
                            mm(ps_w[:, 2 * mt:2 * mt + 2],
                               wvo[:, k * D + mt * 128:k * D + (mt + 1) * 128],
                               sdif_b[:, 2 * k:2 * k + 2],
                               start=(k == 0), stop=(k == ND - 1))
                    nc.vector.tensor_copy(
                        W_sb[:],
                        ps_w[:].rearrange("p (a b) -> p a b", b=2)[:, :, 0])

            if CUT < 2:
                nc.compile()
                return nc
            # ============ phase 2+3: projections interleaved with attn A ====
            attph_ctx = tc.tile_pool(name="attph", bufs=1)
            attph = attph_ctx.__enter__()
            qT = attph.tile([128, ND * S], bf16)
            kT = attph.tile([128, ND * S], bf16)
            E_all = attph.tile([128, ND * S], bf16)
            v64d = persist.tile([128, D], bf16)
            scale = float(1.0 / np.sqrt(DP))
            att_ctx = tc.tile_pool(name="att", bufs=1)
            att = att_ctx.__enter__()
            scA_ctx = tc.tile_pool(name="scA", bufs=3)
            scA = scA_ctx.__enter__()
            with (
                tc.tile_pool(name="psPr", bufs=2, space="PSUM") as psPr,
                tc.tile_pool(name="psA", bufs=1, space="PSUM") as psA,
            ):
                psv = psPr.tile([128, 512], f32, tag="v64")
                for half in range(2):
                    for k in range(ND):
                        mm(psv[half * 64:half * 64 + 64, :],
                           seasT[:, k * S:k * S + 64],
                           wvo[:, k * D:(k + 1) * D],
                           start=(k == 0), stop=(k == ND - 1))
                nc.scalar.copy(v64d[:], psv[:])
                for p in range(4):
                    for wi, (dstt, vc) in enumerate(((qT, VC_BQ), (kT, VC_BK))):
                        for nn in range(2):
                            ps = psPr.tile([128, 512], f32, tag="proj")
                            for k in range(ND):
                                mm(ps[:],
                                   wqk[:, wi * ND * D + k * D + p * 128:
                                       wi * ND * D + k * D + (p + 1) * 128],
                                   seasT[:, k * S + nn * 512:k * S + (nn + 1) * 512],
                                   start=(k == 0), stop=(k == ND - 1))
                            o = dstt[:, p * S + nn * 512:p * S + (nn + 1) * 512]
                            if nn == 0:
                                nc.scalar.activation(
                                    o, ps[:], Act.Identity,
                                    bias=vec[:, vc + p:vc + p + 1], scale=1.0)
                            else:
                                nc.vector.tensor_scalar(
                                    o, ps[:], vec[:, vc + p:vc + p + 1],
                                    None, Alu.add)
                    if CUT < 3:
                        continue
                    for h in (2 * p, 2 * p + 1):
                        ro = 64 * (h % 2)
                        for nn in range(2):
                            c0 = p * S + nn * 512
                            qh = qT[ro:ro + 64, c0:c0 + 512]
                            kh = kT[ro:ro + 64, c0:c0 + 512]
                            qf = psA.tile([128, 512], f32, tag="qf")
                            kf = psA.tile([128, 512], f32, tag="kf")
                            ks = psA.tile([128, 512], f32, tag="ks")
                            mm(qf[:], fwd[ro:ro + 64, 0:128], qh)
                            mm(kf[:], fwd[ro:ro + 64, 0:128], kh)
                            mm(ks[:], fwd[ro:ro + 64, 128:256], kh)
                            qfs = scA.tile([128, 512], bf16, tag="qfs")
                            if nn == 0:
                                nc.scalar.copy(qfs[:], qf[:])
                            else:
                                nc.vector.tensor_copy(qfs[:], qf[:])
                            p1 = scA.tile([128, 512], bf16, tag="p1")
                            p2 = scA.tile([128, 512], bf16, tag="p2")
                            nc.vector.tensor_tensor(p1[:], qfs[:], kf[:], Alu.mult)
                            nc.vector.tensor_tensor(p2[:], qfs[:], ks[:], Alu.mult)
                            cr = psA.tile([128, 512], f32, tag="cr")
                            crs = cr[ro:ro + 64, :]
                            mm(crs, inv[:, 0:64], p1[:], start=True, stop=False)
                            mm(crs, inv[:, 64:128], p2[:], start=False, stop=True)
                            nc.scalar.activation(
                                E_all[ro:ro + 64, c0:c0 + 512], crs,
                                Act.Exp, bias=0.0, scale=scale)

            if DEBUG == 'qT':
                dump(tc, qT, ND * S)
            if CUT < 3:
                scA_ctx.__exit__(None, None, None)
                att_ctx.__exit__(None, None, None)
                attph_ctx.__exit__(None, None, None)
                nc.compile()
                return nc
            if True:
                if DEBUG == 'E':
                    dump(tc, E_all, ND * S)
                # block-diag v tiles per head pair
                bds = []
                for p in range(4):
                    bd = att.tile([128, 128], bf16, tag=f"bd{p}")
                    nc.vector.memset(bd[:], 0.0)
                    nc.vector.tensor_copy(
                        bd[0:64, 0:64], v64d[0:64, (2 * p) * 64:(2 * p + 1) * 64])
                    nc.vector.tensor_copy(
                        bd[64:128, 64:128],
                        v64d[64:128, (2 * p + 1) * 64:(2 * p + 2) * 64])
                    bds.append(bd)

                # Z for all heads
                zinv_b = att.tile([8, S], bf16)
                with tc.tile_pool(name="psZ", bufs=1, space="PSUM") as psZ:
                    Zall = psZ.tile([8, S], f32)
                    for nn in range(2):
                        for p in range(4):
                            mm(Zall[:, nn * 512:(nn + 1) * 512],
                               zsel[:, p * 8:(p + 1) * 8],
                               E_all[:, p * S + nn * 512:p * S + (nn + 1) * 512],
                               start=(p == 0), stop=(p == 3))
                    zsum = att.tile([8, S], f32)
                    nc.vector.tensor_scalar(zsum[:], Zall[:], float(S - DP),
                                            None, Alu.add)
                    zln = att.tile([8, S], f32)
                    nc.scalar.activation(zln[:], zsum[:], Act.Ln, bias=0.0,
                                         scale=1.0)
                    nc.scalar.activation(zinv_b[:], zln[:], Act.Exp, bias=0.0,
                                         scale=-1.0)

                with tc.tile_pool(name="psB", bufs=3, space="PSUM") as psB:
                    for p in range(4):
                        for nn in range(2):
                            sl = slice(nn * 512, (nn + 1) * 512)
                            nv = psB.tile([128, 512], f32, tag="nv")
                            zbc = psB.tile([128, 512], f32, tag="zbc")
                            mm(nv[:], bds[p][:],
                               E_all[:, p * S + nn * 512:p * S + (nn + 1) * 512])
                            mm(zbc[:], hsel[:, p * 128:(p + 1) * 128],
                               zinv_b[:, sl])
                            tmp = scA.tile([128, 512], bf16, tag="nw")
                            nc.scalar.activation(tmp[:], nv[:], Act.Identity,
                                                 bias=W_sb[:, p:p + 1], scale=1.0)
                            nc.vector.tensor_tensor(
                                attnT[:, p * S + nn * 512:p * S + (nn + 1) * 512],
                                tmp[:], zbc[:], Alu.mult)

            scA_ctx.__exit__(None, None, None)
            att_ctx.__exit__(None, None, None)
            if DEBUG == 'E':
                attph_ctx.__exit__(None, None, None)
                nc.compile()
                return nc
            if DEBUG == 'attnT':
                dump(tc, attnT, ND * S)
                attph_ctx.__exit__(None, None, None)
                nc.compile()
                return nc
            attph_ctx.__exit__(None, None, None)

            # ============ back end ==========================================
            with (
                tc.tile_pool(name="bk", bufs=1) as bk,
                tc.tile_pool(name="sc3", bufs=3) as sc3,
                tc.tile_pool(name="sc3s", bufs=1) as sc3s,
                tc.tile_pool(name="psbig", bufs=4, space="PSUM") as psbig,
                tc.tile_pool(name="psst", bufs=1, space="PSUM") as psst,
            ):
                # g3/be3 broadcast tiles (f32)
                g3bc = sc3s.tile([128, D], bf16, tag="g3bc")
                be3bc = sc3s.tile([128, D], bf16, tag="be3bc")
                pg = psbig.tile([128, D], f32, tag="big")
                mm(pg[:, 0:D], r(onesf[:]), r(rowf[:, 0:D]))
                nc.scalar.copy(g3bc[:], pg[:, 0:D])
                pg2 = psbig.tile([128, D], f32, tag="big")
                mm(pg2[:, 0:D], r(onesf[:]), r(rowf[:, D:2 * D]))
                nc.scalar.copy(be3bc[:], pg2[:, 0:D])

                stat3 = sc3s.tile([128, NT], f32, tag="st3")
                statq3 = sc3s.tile([128, NT], f32, tag="st3q")
                rstd3 = sc3s.tile([128, NT], f32, tag="st3r")
                nb3 = sc3s.tile([128, NT], f32, tag="st3n")

                tl = []
                for hf in range(2):
                    t_o1pre = bk.tile([128, ND * 512], bf16, tag=f"o1pre{hf}")
                    t_out1 = bk.tile([128, ND * 512], bf16, tag=f"out1{hf}")
                    t_y = t_o1pre
                    t_yh = bk.tile([128, ND * 512], bf16, tag=f"yh{hf}")
                    t_hT = bk.tile([128, NH * 512], bf16, tag=f"hT{hf}")
                    t_xo = bk.tile([128, ND * 512], f32, tag=f"xo{hf}")
                    tl.append(dict(o1pre=t_o1pre, out1=t_out1, y=t_y,
                                   yh=t_yh, hT=t_hT, xo=t_xo))

                def layernorm_fm(src, dst, gc, bec, sfx):
                    """Feature-major LN over a 512-token half."""
                    s1 = psst.tile([1, 512], f32, tag="s1")
                    s2 = psst.tile([1, 512], f32, tag="s2")
                    for k in range(ND):
                        sq = sc3.tile([128, 512], bf16, tag="lnsq")
                        nc.vector.tensor_tensor(sq[:], src[:, k * 512:(k + 1) * 512],
                                                src[:, k * 512:(k + 1) * 512],
                                                Alu.mult)
                        mm(s1[:], onesb[:, 0:1], src[:, k * 512:(k + 1) * 512],
                           start=(k == 0), stop=(k == ND - 1))
                        mm(s2[:], onesb[:, 0:1], sq[:],
                           start=(k == 0), stop=(k == ND - 1))
                    mean = sc3s.tile([1, 512], f32, tag="lnm" + sfx)
                    msq = sc3s.tile([1, 512], f32, tag="lnq" + sfx)
                    var = sc3s.tile([1, 512], f32, tag="lnv" + sfx)
                    sd = sc3s.tile([1, 512], f32, tag="lnsd" + sfx)
                    rstd_b = sc3s.tile([1, 512], bf16, tag="lnr" + sfx)
                    bb_b = sc3s.tile([1, 512], bf16, tag="lnb" + sfx)
                    nc.vector.tensor_scalar(mean[:], s1[:], 1.0 / D, None,
                                            Alu.mult)
                    nc.vector.tensor_scalar(var[:], s2[:], 1.0 / D, EPS,
                                            Alu.mult, Alu.add)
                    nc.vector.tensor_tensor(msq[:], mean[:], mean[:], Alu.mult)
                    nc.vector.tensor_tensor(var[:], var[:], msq[:], Alu.subtract)
                    nc.scalar.activation(sd[:], var[:], Act.Ln, bias=0.0,
                                         scale=1.0)
                    nc.scalar.activation(rstd_b[:], sd[:], Act.Exp, bias=0.0,
                                         scale=-0.5)
                    nc.vector.tensor_scalar(msq[:], mean[:], -1.0, None,
                                            Alu.mult)
                    nc.vector.tensor_tensor(bb_b[:], msq[:], rstd_b[:], Alu.mult)
                    pa = psst.tile([128, 512], f32, tag="lnA")
                    pb = psst.tile([128, 512], f32, tag="lnB")
                    mm(pa[:], onesb[0:1, :], rstd_b[:])
                    mm(pb[:], onesb[0:1, :], bb_b[:])
                    abc = sc3s.tile([128, 512], bf16, tag="lnabc" + sfx)
                    bbc = sc3s.tile([128, 512], bf16, tag="lnbbc" + sfx)
                    nc.scalar.copy(abc[:], pa[:])
                    nc.scalar.copy(bbc[:], pb[:])
                    for k in range(ND):
                        t = sc3.tile([128, 512], bf16, tag="lnt")
                        nc.vector.tensor_tensor(t[:], src[:, k * 512:(k + 1) * 512],
                                                abc[:], Alu.mult)
                        if gc is None:
                            nc.vector.tensor_tensor(dst[:, k * 512:(k + 1) * 512],
                                                    t[:], bbc[:], Alu.add)
                        else:
                            nc.vector.tensor_tensor(t[:], t[:], bbc[:], Alu.add)
                            nc.scalar.activation(
                                dst[:, k * 512:(k + 1) * 512], t[:],
                                Act.Identity, bias=vec[:, bec + k:bec + k + 1],
                                scale=vec[:, gc + k:gc + k + 1])

                def st_wo(hf):
                    h0 = hf * 512
                    o1pre = tl[hf]["o1pre"]
                    for mt in range(ND):
                        ps = psbig.tile([128, 512], f32, tag="big")
                        for k in range(ND):
                            mm(ps[:],
                               wvo[:, ND * D + k * D + mt * 128:
                                   ND * D + k * D + (mt + 1) * 128],
                               attnT[:, k * S + h0:k * S + h0 + 512],
                               start=(k == 0), stop=False)
                        mm(ps[:], identb[:],
                           seasT[:, mt * S + h0:mt * S + h0 + 512],
                           start=False, stop=True)
                        o = o1pre[:, mt * 512:(mt + 1) * 512]
                        if mt % 2 == 0:
                            nc.scalar.activation(
                                o, ps[:], Act.Identity,
                                bias=vec[:, VC_BO + mt:VC_BO + mt + 1], scale=1.0)
                        else:
                            nc.vector.tensor_scalar(
                                o, ps[:], vec[:, VC_BO + mt:VC_BO + mt + 1],
                                None, Alu.add)

                def st_ln1(hf):
                    layernorm_fm(tl[hf]["o1pre"], tl[hf]["out1"],
                                 VC_G1, VC_BE1, str(hf))

                def st_ffn1(hf):
                    out1, hT = tl[hf]["out1"], tl[hf]["hT"]
                    for mt in range(NH):
                        ps = psbig.tile([128, 512], f32, tag="big")
                        for k in range(ND):
                            mm(ps[:],
                               w1_s[:, k * DH + mt * 128:k * DH + (mt + 1) * 128],
                               out1[:, k * 512:(k + 1) * 512],
                               start=(k == 0), stop=(k == ND - 1))
                        o = hT[:, mt * 512:(mt + 1) * 512]
                        if mt % 2 == 0:
                            nc.scalar.activation(
                                o, ps[:], Act.Relu,
                                bias=vec[:, VC_B1 + mt:VC_B1 + mt + 1], scale=1.0)
                        else:
                            nc.vector.tensor_scalar(
                                o, ps[:], vec[:, VC_B1 + mt:VC_B1 + mt + 1],
                                0.0, Alu.add, Alu.max)

                def st_ffn2(hf):
                    out1, hT, y = tl[hf]["out1"], tl[hf]["hT"], tl[hf]["y"]
                    for mt in range(ND):
                        ps = psbig.tile([128, 512], f32, tag="big")
                        for k in range(NH):
                            mm(ps[:],
                               w2_s[:, k * D + mt * 128:k * D + (mt + 1) * 128],
                               hT[:, k * 512:(k + 1) * 512],
                               start=(k == 0), stop=False)
                        mm(ps[:], identb[:], out1[:, mt * 512:(mt + 1) * 512],
                           start=False, stop=True)
                        o = y[:, mt * 512:(mt + 1) * 512]
                        if mt % 2 == 0:
                            nc.scalar.activation(
                                o, ps[:], Act.Identity,
                                bias=vec[:, VC_B2 + mt:VC_B2 + mt + 1], scale=1.0)
                        else:
                            nc.vector.tensor_scalar(
                                o, ps[:], vec[:, VC_B2 + mt:VC_B2 + mt + 1],
                                None, Alu.add)

                def st_ln2(hf):
                    layernorm_fm(tl[hf]["y"], tl[hf]["yh"], None, None, str(hf))

                def st_tailmm(hf):
                    yh, xo = tl[hf]["yh"], tl[hf]["xo"]
                    for si in range(4):
                        st = hf * 4 + si
                        so = si * 128
                        pst = psbig.tile([128, 512], f32, tag="big")
                        for kt in range(ND):
                            mm(pst[:, kt * 128:(kt + 1) * 128],
                               yh[:, kt * 512 + so:kt * 512 + so + 128],
                               g2diag[:, kt * 128:(kt + 1) * 128],
                               start=(kt == 0), stop=False)
                        mm(pst[:], identb[:], x_tok[:, st * D:(st + 1) * D],
                           start=False, stop=False)
                        mm(pst[:], onesb[0:1, :], rowb[:], start=False,
                           stop=True)
                        xs = xo[:, si * 512:(si + 1) * 512]
                        with nc.allow_low_precision(reason="accum copy"):
                            nc.scalar.activation(
                                xs, pst[:], Act.Copy,
                                accum_out=stat3[:, st:st + 1])
                        ttd = sc3.tile([128, D], bf16, tag="ttd")
                        nc.gpsimd.tensor_tensor(ttd[:], xs, xs, Alu.mult)
                        nc.vector.tensor_reduce(statq3[:, st:st + 1], ttd[:],
                                                mybir.AxisListType.X, Alu.add)

                def st_tailmath(hf):
                    hs = slice(hf * 4, hf * 4 + 4)
                    nc.vector.tensor_scalar(stat3[:, hs], stat3[:, hs],
                                            1.0 / D, None, Alu.mult)
                    nc.vector.tensor_scalar(statq3[:, hs], statq3[:, hs],
                                            1.0 / D, EPS, Alu.mult, Alu.add)
                    nc.vector.tensor_tensor(rstd3[:, hs], stat3[:, hs],
                                            stat3[:, hs], Alu.mult)
                    nc.vector.tensor_tensor(statq3[:, hs], statq3[:, hs],
                                            rstd3[:, hs], Alu.subtract)
                    nc.scalar.activation(rstd3[:, hs], statq3[:, hs],
                                         Act.Ln, bias=0.0, scale=1.0)
                    nc.scalar.activation(rstd3[:, hs], rstd3[:, hs],
                                         Act.Exp, bias=0.0, scale=-0.5)
                    nc.vector.tensor_tensor(nb3[:, hs], stat3[:, hs],
                                            rstd3[:, hs], Alu.mult)
                    nc.vector.tensor_scalar(nb3[:, hs], nb3[:, hs],
                                            -1.0, None, Alu.mult)

                def st_tailout(hf):
                    xo = tl[hf]["xo"]
                    for si in range(4):
                        st = hf * 4 + si
                        xs = xo[:, si * 512:(si + 1) * 512]
                        xh = sc3.tile([128, D], bf16, tag="xh")
                        nc.scalar.activation(xh[:], xs, Act.Identity,
                                             bias=nb3[:, st:st + 1],
                                             scale=rstd3[:, st:st + 1])
                        nc.vector.tensor_tensor(xh[:], xh[:], g3bc[:], Alu.mult)
                        xn = sc3.tile([128, D], f32, tag="xn")
                        nc.vector.tensor_tensor(xn[:], xh[:], be3bc[:], Alu.add)
                        nc.sync.dma_start(out_d[st * 128:(st + 1) * 128, :],
                                          xn[:])

                if DEBUG is None:
                    st_wo(0); st_ln1(0); st_wo(1); st_ln1(1)
                    st_ffn1(0); st_ffn1(1)
                    st_ffn2(0); st_ln2(0); st_ffn2(1)
                    st_tailmm(0); st_tailmath(0); st_tailout(0)
                    st_ln2(1); st_tailmm(1)
                    st_tailmath(1); st_tailout(1)
                else:
                    for hf in range(2):
                        st_wo(hf)
                        if DEBUG == 'o1pre':
                            dump(tc, tl[hf]["o1pre"], ND * 512, aoff=4 * hf)
                            continue
                        st_ln1(hf)
                        if DEBUG == 'out1':
                            dump(tc, tl[hf]["out1"], ND * 512, aoff=4 * hf)
                            continue
                        st_ffn1(hf)
                        st_ffn2(hf)
                        if DEBUG == 'y':
                            dump(tc, tl[hf]["y"], ND * 512, aoff=4 * hf)
                            continue
                        st_ln2(hf)
                        if DEBUG == 'yh':
                            dump(tc, tl[hf]["yh"], ND * 512, aoff=4 * hf)
                            continue
                        st_tailmm(hf)
                        if DEBUG == 'xo':
                            for si in range(4):
                                st = hf * 4 + si
                                nc.sync.dma_start(
                                    out_d[st * 128:(st + 1) * 128, :],
                                    tl[hf]["xo"][:, si * 512:(si + 1) * 512])
                            continue
                        st_tailmath(hf)
                        st_tailout(hf)

    nc.compile()
    return nc


def _get_nc():
    if "nc" not in _CACHE:
        _CACHE["nc"] = _build()
    return _CACHE["nc"]


def _col(v):
    return np.ascontiguousarray(
        np.asarray(v, np.float32).reshape(-1, 128).T)


def make_in_maps(inputs):
    f32 = np.float32
    g = {k: np.asarray(v, f32) for k, v in inputs.items() if k != "num_heads"}

    def prep_w(w, nk):
        return np.ascontiguousarray(
            np.asarray(w, BF).reshape(nk, 128, -1).transpose(1, 0, 2)
            .reshape(128, -1))

    wqk = np.concatenate([prep_w(g["wq"], 4), prep_w(g["wk"], 4)], axis=1)
    wvo = np.concatenate([prep_w(g["wv"], 4), prep_w(g["wo"], 4)], axis=1)
    w1p = prep_w(g["w1"], 4)
    w2p = prep_w(g["w2"], 16)
    bo_p = g["bo"] + g["bv"] @ g["wo"]
    vecs = np.zeros((128, VC_N), f32)
    vecs[:, VC_BQ:VC_BQ + 4] = _col(g["bq"])
    vecs[:, VC_BK:VC_BK + 4] = _col(g["bk"])
    vecs[:, VC_BO:VC_BO + 4] = _col(bo_p)
    vecs[:, VC_B2:VC_B2 + 4] = _col(g["b2"])
    vecs[:, VC_G1:VC_G1 + 4] = _col(g["g1"])
    vecs[:, VC_BE1:VC_BE1 + 4] = _col(g["be1"])
    vecs[:, VC_G2:VC_G2 + 4] = _col(g["g2"])
    vecs[:, VC_BE2:VC_BE2 + 4] = _col(g["be2"])
    vecs[:, VC_B1:VC_B1 + 16] = _col(g["b1"])
    rowsf = np.concatenate([g["g3"], g["be3"]])[None, :].astype(f32)
    rowsb = np.ascontiguousarray(g["be2"][None, :].astype(BF))
    g2diag = np.zeros((128, D), BF)
    for kt in range(ND):
        g2diag[:, kt * 128:(kt + 1) * 128] = np.diag(
            g["g2"][kt * 128:(kt + 1) * 128]).astype(BF)
    shared = dict(wqk=wqk, wvo=wvo, w1p=w1p, w2p=w2p, vecs=vecs,
                  rowsf=rowsf, rowsb=rowsb, g2diag=g2diag)
    shared = {k: np.ascontiguousarray(v) for k, v in shared.items()}
    x = np.asarray(inputs["x"], f32)
    return [dict(shared, xb=np.ascontiguousarray(x[b].astype(BF)))
            for b in range(NCORES)]


def kernel(**inputs):
    from concourse.bass_utils import run_bass_kernel_spmd

    nc = _get_nc()
    in_maps = make_in_maps(inputs)
    res = run_bass_kernel_spmd(nc, in_maps, list(range(NCORES)))
    out = np.stack([res.results[b]["out"] for b in range(NCORES)], axis=0)
    return out.astype(np.float32)
